# revision 4
# baseline (speedup 1.0000x reference)
"""Trainium2 Bass kernel for nn_Attention_84567906058480 (optimized).

Multi-head attention (B=4, T=2048, C=1024, H=16, D=64) on 8 NeuronCores.
Core c = (batch b = c//2, head-group hg = c%2).  HW exec ~415 us
(baseline 507 us).

Design (evidence from perfetto/ntff traces):
- The attention inner loop is EXP-limited: each kp-iter needs one ACT-engine
  exp over [128, 1024] (~1.04 us) vs ~0.85 us of PE matmul work; total exp
  floor is ~266 us and sets the attention-phase length.  Everything else is
  scheduled around keeping the ACT engine saturated and the PE busy with
  exp-independent work.
- Host-transposed xT [C, T] is DMA'd directly (no PE transposes); xT arrives
  as per-(ct, tch) 128 KB pieces, tch-major, so the first K-projection
  chains are PE-bound almost immediately.
- Phase B: K, Q, V projections (bf16 PE, f32 PSUM accumulate), all
  PSUM->SBUF copies on DVE.
- Attention: segments (qc0 h0-7), (qc2 ...), (qc1 ...), (qc3 ...), emitted
  as ONE continuous software pipeline: scores+exp run LAG=3 kp-iters ahead
  of the attn-out matmuls, flowing through segment boundaries without
  draining; the softmax denominator comes from a ones-column appended to V;
  normalization = DVE l_pad copy + Emat broadcast matmul + DVE fast
  reciprocal + DVE multiply (no ACT work besides the exps).
- Output projection emits CHANNEL-major partials (stationary woT tile,
  moving lout): pj [128ch, 512tok], bias via per-partition tensor_scalar
  add, emitted as 2-matmul filler bursts (under the exp slack) during the
  next q-chunk's attention.  The pairwise ReduceScatter splits CHANNELS
  (even core keeps ch 0-511, odd 512-1023, for all its tokens), so each RS
  chunk depends on a single q-chunk and overlaps the following attention;
  only qc3's chunk (~1 MB RS, ~20 us) remains in the tail.  The RS result
  is DMA'd DRAM->DRAM into the bf16 output (no on-chip drain/cast).
- out: [4 * C/2, 512] bf16, qc-major channel blocks; the host transposes,
  concatenates, and converts to f32.
"""

import os
import sys
import types
import contextlib

import numpy as np

if "/opt/trn_rl_repo" not in sys.path:
    sys.path.insert(0, "/opt/trn_rl_repo")

import ml_dtypes
import concourse.bass as bass  # noqa: F401
import concourse.mybir as mybir
import concourse.tile as tile
from concourse import bacc
from concourse import bass_utils

F32 = mybir.dt.float32
BF16 = mybir.dt.bfloat16
AF = mybir.ActivationFunctionType

B, T, C = 4, 2048, 1024
H, D = 16, 64
HPC = 8            # heads per core
FS = HPC * D       # per-core feature shard = 512
N_CORES = 8
PAIRS = [[0, 1], [2, 3], [4, 5], [6, 7]]

NT = T // 128      # 16 token tiles
NCT = C // 128     # 8 contraction tiles
NFB = FS // 128    # 4 feature blocks per core
QW = 512           # q chunk width
NQC = T // QW      # 4 q chunks


def _emit(nc, tc, xt_ext, wqt_ext, wkt_ext, wvt_ext, wot_ext, bo_ext, out_ext):
    with tc.tile_pool(name="const", bufs=1) as constp, \
         tc.tile_pool(name="persist", bufs=1) as pp, \
         tc.tile_pool(name="pd", bufs=4) as pd, \
         tc.tile_pool(name="pdram", bufs=1, space="DRAM") as pdram, \
         tc.tile_pool(name="ps_acc", bufs=2, space="PSUM") as ps_acc, \
         tc.tile_pool(name="ps_sT", bufs=1, space="PSUM") as ps_sT, \
         tc.tile_pool(name="ps_oT", bufs=2, space="PSUM") as ps_oT:

        # ---- constants -------------------------------------------------
        Emat = constp.tile([128, 64], BF16, tag="Emat")
        nc.gpsimd.memset(Emat[:, :], 0.0)
        nc.gpsimd.memset(Emat[64:65, :], 1.0)
        # bo as per-partition columns: bo_col[p, cc] = bo[cc*128 + p]
        bo_col = constp.tile([128, NCT], F32, tag="bo_col")
        nc.sync.dma_start(bo_col[:, :], bo_ext[:].rearrange("(cc p) -> p cc", p=128))
        # tiny exp to pull the ACT table load into the DMA lead-in
        warm = constp.tile([1, 8], F32, tag="warm")
        nc.gpsimd.memset(warm[:, :], 0.0)
        nc.scalar.activation(warm[:, :], warm[:, :], AF.Exp)

        # ---- persistent activation storage (bf16) ----------------------
        qT = [pp.tile([128, T], BF16, tag=f"qT{fb}", name=f"qT{fb}") for fb in range(NFB)]
        kTh = [pp.tile([128, T], BF16, tag=f"kTh{h}", name=f"kTh{h}") for h in range(HPC)]
        v_ext = [pp.tile([128, HPC * 65], BF16, tag=f"vx{tt}", name=f"vx{tt}") for tt in range(NT)]
        woT = [pp.tile([128, C], BF16, tag=f"woT{fb}", name=f"woT{fb}") for fb in range(NFB)]
        lout = [pp.tile([128, T], BF16, tag=f"lo{fb}", name=f"lo{fb}") for fb in range(NFB)]
        xT = [pp.tile([128, T], BF16, tag=f"xT{ct}", name=f"xT{ct}") for ct in range(NCT)]
        wkTf = pp.tile([128, NCT * FS], BF16, tag="wkTf", name="wkTf")
        wqTf = pp.tile([128, NCT * FS], BF16, tag="wqTf", name="wqTf")
        wvTf = pp.tile([128, NCT * FS], BF16, tag="wvTf", name="wvTf")

        # kTh[h]: head h's k at partitions (h%2)*64..+64, zeros elsewhere
        for h in range(HPC):
            z0 = (1 - (h % 2)) * 64
            nc.gpsimd.memset(kTh[h][z0:z0 + 64, :], 0.0)

        # =================================================================
        # Phase B: DMAs + K/Q/V projections
        # =================================================================
        for ct in range(NCT):
            nc.sync.dma_start(wkTf[:, ct * FS:(ct + 1) * FS],
                              wkt_ext[ct * 128:(ct + 1) * 128, :])
            nc.sync.dma_start(
                xT[ct][:, 0:QW],
                xt_ext[ct * 128:(ct + 1) * 128, 0:QW])
        for tch in range(1, NQC):
            for ct in range(NCT):
                nc.sync.dma_start(
                    xT[ct][:, tch * QW:(tch + 1) * QW],
                    xt_ext[ct * 128:(ct + 1) * 128, tch * QW:(tch + 1) * QW])
        for ct in range(NCT):
            nc.sync.dma_start(wqTf[:, ct * FS:(ct + 1) * FS],
                              wqt_ext[ct * 128:(ct + 1) * 128, :])
        for ct in range(NCT):
            nc.sync.dma_start(wvTf[:, ct * FS:(ct + 1) * FS],
                              wvt_ext[ct * 128:(ct + 1) * 128, :])
        for fb in range(NFB):
            nc.sync.dma_start(woT[fb][:, :], wot_ext[fb * 128:(fb + 1) * 128, :])

        def k_chain(fb, tch):
            acc = ps_acc.tile([128, QW], F32, tag="acc")
            for ct in range(NCT):
                nc.tensor.matmul(
                    acc[:, :],
                    wkTf[:, ct * FS + fb * 128: ct * FS + fb * 128 + 128],
                    xT[ct][:, tch * QW:(tch + 1) * QW],
                    start=(ct == 0), stop=(ct == NCT - 1))
            for hh in range(2):
                nc.vector.tensor_copy(
                    kTh[fb * 2 + hh][hh * 64:(hh + 1) * 64,
                                     tch * QW:(tch + 1) * QW],
                    acc[hh * 64:(hh + 1) * 64, :])

        def q_chain(fb, tch):
            acc = ps_acc.tile([128, QW], F32, tag="acc")
            for ct in range(NCT):
                nc.tensor.matmul(
                    acc[:, :],
                    wqTf[:, ct * FS + fb * 128: ct * FS + fb * 128 + 128],
                    xT[ct][:, tch * QW:(tch + 1) * QW],
                    start=(ct == 0), stop=(ct == NCT - 1))
            nc.vector.tensor_copy(qT[fb][:, tch * QW:(tch + 1) * QW], acc[:, :])

        def v_chain(tt):
            acc = ps_acc.tile([128, FS], F32, tag="acc")
            for ct in range(NCT):
                nc.tensor.matmul(
                    acc[:, :],
                    xT[ct][:, tt * 128:(tt + 1) * 128],
                    wvTf[:, ct * FS:(ct + 1) * FS],
                    start=(ct == 0), stop=(ct == NCT - 1))
            nc.gpsimd.memset(v_ext[tt][:, :], 1.0)
            dst = v_ext[tt][:].rearrange("p (h e) -> p h e", e=65)[:, :, 0:64]
            src = acc[:].rearrange("p (h e) -> p h e", e=64)
            nc.vector.tensor_copy(dst, src)

        for tch in range(NQC):
            for fb in range(NFB):
                k_chain(fb, tch)
        for fb in range(NFB):
            for tch in range(NQC):
                q_chain(fb, tch)
        for tt in range(NT):
            v_chain(tt)

        # =================================================================
        # Phase D: attention + channel-major projection + ReduceScatter
        # =================================================================
        l_pad = pd.tile([128, QW], BF16, tag="l_pad", bufs=1, name="l_pad")
        nc.gpsimd.memset(l_pad[:, :], 0.0)
        sTs = [ps_sT.tile([128, 1024], F32, tag=f"sT{i}", name=f"sT{i}", bufs=1)
               for i in range(2)]

        LAG = 3  # outT matmuls run LAG kp-iterations behind sT/exp
        pending_norm = []
        filler_chains = []   # (emit_callback, qc_chunk) pairs
        chunk_left = {}      # qc -> chains remaining before RS can fire
        rs_done = set()
        phases = os.environ.get("KERNEL_PHASES", "full")

        # per-qc RS input buffers [C, QW] (channels x tokens); the RS writes
        # its [C/2, QW] result directly into the qc's block of the bf16
        # output tensor -- no on-chip drain at all.
        rs_in = {qc: pdram.tile([C, QW], BF16, tag=f"rs_in{qc}", name=f"rs_in{qc}")
                 for qc in range(NQC)}
        rs_out = {qc: pdram.tile([C // 2, QW], BF16, tag=f"rs_out{qc}",
                                 name=f"rs_out{qc}")
                  for qc in range(NQC)}

        def rs_and_drain(qc):
            rs_done.add(qc)
            out_blk = out_ext[qc * (C // 2):(qc + 1) * (C // 2), :]
            if phases == "nors":
                nc.sync.dma_start(rs_out[qc][:, :], rs_in[qc][0:C // 2, :])
            else:
                nc.gpsimd.collective_compute(
                    "ReduceScatter", mybir.AluOpType.add,
                    replica_groups=PAIRS,
                    ins=[rs_in[qc].opt()], outs=[rs_out[qc].opt()])
            nc.sync.dma_start(out_blk, rs_out[qc][:, :])

        def make_proj_chains(qc):
            # 8 chains (one per 128-channel tile), each emitted as two
            # 2-matmul bursts so every PE burst stays under the exp slack
            chains = []
            for cc in range(NCT):
                box = [None]

                def emit_lo(cc=cc, qc=qc, box=box):
                    box[0] = ps_acc.tile([128, QW], F32, tag="acc", name="pj")
                    for fb in (0, 1):
                        nc.tensor.matmul(
                            box[0][:, :],
                            woT[fb][:, cc * 128:(cc + 1) * 128],
                            lout[fb][:, qc * QW:(qc + 1) * QW],
                            start=(fb == 0), stop=False)

                def emit_hi(cc=cc, qc=qc, box=box):
                    for fb in (2, 3):
                        nc.tensor.matmul(
                            box[0][:, :],
                            woT[fb][:, cc * 128:(cc + 1) * 128],
                            lout[fb][:, qc * QW:(qc + 1) * QW],
                            start=False, stop=(fb == NFB - 1))
                    ot = pd.tile([128, QW], BF16, tag="ot")
                    nc.vector.tensor_scalar_add(ot[:, :], box[0][:, :],
                                                bo_col[:, cc:cc + 1])
                    nc.sync.dma_start(rs_in[qc][cc * 128:(cc + 1) * 128, :],
                                      ot[:, :])
                chains.append((emit_lo, qc))
                chains.append((emit_hi, qc))
            return chains

        def emit_filler():
            if filler_chains:
                fn, qc = filler_chains.pop(0)
                fn()
                chunk_left[qc] -= 1
                if chunk_left[qc] == 0:
                    rs_and_drain(qc)

        NKP = NT // 2
        segments = [(h, qc) for qc in (0, 2, 1, 3) for h in range(HPC)]
        NSEG = len(segments)
        outTs = {}
        pTs = {}

        def emit_scores(gi):
            seg, kp = divmod(gi, NKP)
            h, qc = segments[seg]
            sT = sTs[gi % 2]
            q_ap = qT[h // 2][:, qc * QW:(qc + 1) * QW]
            for j in range(2):
                kt = kp * 2 + j
                nc.tensor.matmul(
                    sT[:, j * 512:(j + 1) * 512],
                    kTh[h][:, kt * 128:(kt + 1) * 128],
                    q_ap, start=True, stop=True)
            pT = pd.tile([128, 1024], BF16, tag="pT", bufs=6)
            nc.scalar.activation(pT[:, :], sT[:, :], AF.Exp)
            pTs[gi] = pT

        def make_norm(seg):
            h, qc = segments[seg]
            fb, hh = divmod(h, 2)
            outT = outTs[seg]

            def norm():
                nc.vector.tensor_copy(l_pad[64:65, :], outT[64:65, :])
                rb_ps = ps_acc.tile([128, QW], F32, tag="acc", name="rb_ps")
                nc.tensor.matmul(rb_ps[0:64, :], Emat[:, :], l_pad[:, :],
                                 start=True, stop=True)
                rb_sb = pd.tile([64, QW], F32, tag="rb_sb", bufs=2)
                nc.vector.reciprocal_approx_fast(rb_sb[:, :], rb_ps[0:64, :])
                nc.vector.tensor_mul(
                    lout[fb][hh * 64:(hh + 1) * 64, qc * QW:(qc + 1) * QW],
                    outT[0:64, :], rb_sb[:, :])
                del outTs[seg]
            return norm

        def emit_outT(go):
            seg, okp = divmod(go, NKP)
            h, qc = segments[seg]
            if okp == 0:
                outTs[seg] = ps_oT.tile([65, QW], F32, tag="outT",
                                        name=f"outT{seg}")
            outT = outTs[seg]
            pT = pTs.pop(go)
            for j in range(2):
                kt = okp * 2 + j
                nc.tensor.matmul(
                    outT[:, :],
                    v_ext[kt][:, h * 65:(h + 1) * 65],
                    pT[:, j * 512:(j + 1) * 512],
                    start=(okp == 0 and j == 0),
                    stop=(okp == NKP - 1 and j == 1))
            if okp == NKP - 1:
                pending_norm.append(make_norm(seg))
                # once the last head of a qc finishes, queue its proj chains
                if (seg + 1) % HPC == 0:
                    qcd = segments[seg][1]
                    chains = make_proj_chains(qcd)
                    chunk_left[qcd] = len(chains)
                    filler_chains.extend(chains)

        if phases == "qkv":
            dbg = pd.tile([128, QW], BF16, tag="dbg", bufs=1)
            nc.vector.tensor_copy(dbg[:, :], qT[0][:, 0:QW])
            nc.sync.dma_start(out_ext[0:128, 0:QW], dbg[:, :])
            return

        # global software pipeline: scores/exp LAG iters ahead of attn-out,
        # flowing through segment boundaries without draining
        for gi in range(NSEG * NKP):
            kp = gi % NKP
            emit_scores(gi)
            if kp == 5 and pending_norm:
                pending_norm.pop(0)()
            if kp in (2, 3, 6, 7):
                emit_filler()
            if gi >= LAG:
                emit_outT(gi - LAG)
        for go in range(NSEG * NKP - LAG, NSEG * NKP):
            emit_outT(go)
        while pending_norm:
            pending_norm.pop(0)()
        while filler_chains:
            emit_filler()
        for qc in range(NQC):
            if qc not in rs_done:
                rs_and_drain(qc)


def _build_nc():
    nc = bacc.Bacc("TRN2", target_bir_lowering=False, debug=False,
                   num_devices=N_CORES)
    xt_ext = nc.dram_tensor("xt", [C, T], BF16, kind="ExternalInput")
    wqt_ext = nc.dram_tensor("wqt", [C, FS], BF16, kind="ExternalInput")
    wkt_ext = nc.dram_tensor("wkt", [C, FS], BF16, kind="ExternalInput")
    wvt_ext = nc.dram_tensor("wvt", [C, FS], BF16, kind="ExternalInput")
    wot_ext = nc.dram_tensor("wot", [FS, C], BF16, kind="ExternalInput")
    bo_ext = nc.dram_tensor("bo", [C], F32, kind="ExternalInput")
    out_ext = nc.dram_tensor("out", [NQC * (C // 2), QW], BF16,
                             kind="ExternalOutput")
    with tile.TileContext(nc) as tc:
        _emit(nc, tc, xt_ext, wqt_ext, wkt_ext, wvt_ext, wot_ext, bo_ext, out_ext)
    nc.finalize()
    return nc


# ---------------------------------------------------------------------------
# NTFF profiling under axon (used when KERNEL_TRACE=1)
# ---------------------------------------------------------------------------
def _ensure_axon_hooks():
    try:
        from antenv.axon_hooks import get_axon_ntff_profile_hook  # noqa: F401
        return
    except ImportError:
        pass
    import ctypes
    import antenv

    so_path = "/opt/axon/libaxon_pjrt.so"
    lib = ctypes.CDLL(so_path)
    if not hasattr(lib, "axon_start_nrt_profile"):
        return
    lib.axon_start_nrt_profile.argtypes = [ctypes.POINTER(ctypes.c_int64),
                                           ctypes.c_size_t]
    lib.axon_start_nrt_profile.restype = ctypes.c_int64
    lib.axon_stop_nrt_profile.argtypes = [ctypes.c_char_p]
    lib.axon_stop_nrt_profile.restype = ctypes.c_int64

    @contextlib.contextmanager
    def _hook(output_dir, device_ids):
        import jax
        jax.devices()
        if device_ids:
            ids = (ctypes.c_int64 * len(device_ids))(*device_ids)
            rc = lib.axon_start_nrt_profile(ids, len(device_ids))
        else:
            rc = lib.axon_start_nrt_profile(None, 0)
        if rc != 0:
            raise RuntimeError(f"axon_start_nrt_profile rc={rc}")
        try:
            yield
        finally:
            n = lib.axon_stop_nrt_profile(str(output_dir).encode())
            print(f"ntff profile: {n} file(s) -> {output_dir}", file=sys.stderr)

    holder = [_hook]
    mod = types.ModuleType("antenv.axon_hooks")
    mod.get_axon_ntff_profile_hook = lambda: holder[0]
    mod.set_axon_ntff_profile_hook = lambda h: holder.__setitem__(0, h)
    sys.modules["antenv.axon_hooks"] = mod
    antenv.axon_hooks = mod
    bass_utils.upload_artifacts = lambda tmpdir: f"(local:{tmpdir})"


_NC = None
LAST = {}


def kernel(hidden_states, wq, wk, wv, wo, bo):
    global _NC
    hidden_states = np.asarray(hidden_states, dtype=np.float32)
    wq = np.asarray(wq, dtype=np.float32)
    wk = np.asarray(wk, dtype=np.float32)
    wv = np.asarray(wv, dtype=np.float32)
    wo = np.asarray(wo, dtype=np.float32)
    bo = np.asarray(bo, dtype=np.float32)

    if _NC is None:
        _NC = _build_nc()

    bf = ml_dtypes.bfloat16
    scale = np.float32(D ** -0.5)
    in_maps = []
    for c in range(N_CORES):
        b, hg = divmod(c, 2)
        fr = hg * FS
        in_maps.append({
            "xt": np.ascontiguousarray(hidden_states[b].T).astype(bf),
            "wqt": np.ascontiguousarray((wq[fr:fr + FS] * scale).T).astype(bf),
            "wkt": np.ascontiguousarray(wk[fr:fr + FS].T).astype(bf),
            "wvt": np.ascontiguousarray(wv[fr:fr + FS].T).astype(bf),
            "wot": np.ascontiguousarray(wo[:, fr:fr + FS].T).astype(bf),
            "bo": bo * np.float32(0.5),
        })

    trace = os.environ.get("KERNEL_TRACE", "0") == "1"
    if trace:
        _ensure_axon_hooks()
    res = bass_utils.run_bass_kernel_spmd(
        _NC, in_maps, core_ids=list(range(N_CORES)), trace=trace)
    LAST["exec_time_ns"] = res.exec_time_ns
    LAST["res"] = res

    y = np.empty((B, T, C), dtype=np.float32)
    for c in range(N_CORES):
        b, hg = divmod(c, 2)
        blk = np.asarray(res.results[c]["out"]).astype(np.float32)
        for qc in range(NQC):
            y[b, qc * QW:(qc + 1) * QW, hg * (C // 2):(hg + 1) * (C // 2)] = \
                blk[qc * (C // 2):(qc + 1) * (C // 2), :].T
    return y
